# revision 11
# baseline (speedup 1.0000x reference)
"""VQ codebook-lookup kernel for TRN2, data-parallel over batch on 8 NeuronCores.

Reference computation (per batch b with class c[b]):
  z = z_e_x[b] viewed as [N=4096, D=256] (D innermost)
  cb = embedding[c[b]*512:(c[b]+1)*512]            # [K=512, D]
  idx[n] = argmin_k ||z[n] - cb[k]||^2 = argmax_k (z[n].cb[k] - ||cb[k]||^2/2)
  out[n] = cb[idx[n]]

Device strategy per core (4 batches):
  - scores S[n,k] on TensorE, all at a common 2^18 product scale (argmax is
    scale-invariant):
      * main term z1.c1 in float32r (the PE keeps 11 mantissa bits of f32r
        operands, measured; operands are pre-rounded to 11 bits on the host
        so the pass is exact) — 2 matmuls over the d=256 contraction
      * cross corrections z2.c1 + z1.c2 in one fp8-e4m3 DoubleRow pair per
        PE cell — 2 matmuls; one DR slot is stolen to add the bias low part
      * bias high part (r2n11 of -2^18*||cb||^2/2) via a contraction-1
        f32r matmul that starts the PSUM accumulation group
  - row-max on DVE straight from PSUM, one-hot = (S == max) on DVE
  - one-hot transposed on TensorE, then a bf16 matmul gathers the codewords
Host side only reindexes/splits/quantizes operands and reassembles the
output.  Simulated end-to-end argmin flips vs the f32 reference: 2 of 131072
(the exact-f32 baseline had 4).
"""

import sys

sys.path.insert(0, "/opt/trn_rl_repo")

import numpy as np

B, D, HH, WW = 32, 256, 64, 64
N = HH * WW            # 4096 positions per batch
K = 512                # codes per class
NUM_CLASSES = 60
NCORES = 8
BPC = B // NCORES      # batches per core
NT = N // 128          # 32 n-tiles per batch

_CACHE = {}

# set by test harness to request an NTFF profile
TRACE = False
LAST_EXEC_NS = None


def _build(bpc=BPC, nt=NT, repeat=1, psum_cfg=(3, 1, 1), sco_bufs=3, zb_bufs=2):
    from concourse import bacc, tile, mybir
    import ml_dtypes

    f32 = mybir.dt.float32
    f32r = mybir.dt.float32r
    f16 = mybir.dt.float16
    bf16 = mybir.dt.bfloat16
    Alu = mybir.AluOpType

    nc = bacc.Bacc("TRN2", target_bir_lowering=False)

    fp8 = mybir.dt.float8e4
    z_ext = nc.declare_dram_parameter("z", [bpc, 128, 2, N], f16, isOutput=False)
    ct_ext = nc.declare_dram_parameter("ct", [bpc, 128, 2, K], f16, isOutput=False)
    zdr_ext = nc.declare_dram_parameter("zdr", [bpc, 128, 2, 2, N], fp8,
                                        isOutput=False)
    cdr_ext = nc.declare_dram_parameter("cdr", [bpc, 128, 2, 2, K], fp8,
                                        isOutput=False)
    cbgb_ext = nc.declare_dram_parameter("cbgb", [bpc, 128, 4, D], bf16,
                                         isOutput=False)
    biasrow_ext = nc.declare_dram_parameter("biasrow", [bpc, 1, K], f32r,
                                            isOutput=False)
    ones_ext = nc.declare_dram_parameter("ones", [1, 128], f32r, isOutput=False)
    out_ext = nc.declare_dram_parameter("out", [bpc, 128, nt, D], bf16, isOutput=True)

    ident_dram = nc.inline_tensor(np.eye(128, dtype=ml_dtypes.bfloat16), name="ident")

    with tile.TileContext(nc) as tc:
        with (
            tc.tile_pool(name="const", bufs=1) as constp,
            tc.tile_pool(name="zb", bufs=zb_bufs) as zb,
            tc.tile_pool(name="cbp", bufs=2) as cbp,
            tc.tile_pool(name="outp", bufs=2) as outp,
            tc.tile_pool(name="sco", bufs=sco_bufs) as sco,
            tc.tile_pool(name="psS", bufs=psum_cfg[0], space="PSUM") as psSp,
            tc.tile_pool(name="psT", bufs=psum_cfg[1], space="PSUM") as psTp,
            tc.tile_pool(name="psQ", bufs=psum_cfg[2], space="PSUM") as psQp,
        ):
            ident = constp.tile([128, 128], bf16, tag="ident")
            ones = constp.tile([1, 128], f32r, tag="ones")
            nc.sync.dma_start(ident[:], ident_dram[:])
            nc.sync.dma_start(ones[:], ones_ext[:])

            for b in [bb for _ in range(repeat) for bb in range(bpc)]:
                z = zb.tile([128, 2, N], f16, tag="z")
                zdr = zb.tile([128, 2, 2, N], fp8, tag="zdr")
                ct = cbp.tile([128, 2, K], f16, tag="ct")
                cdr = cbp.tile([128, 2, 2, K], fp8, tag="cdr")
                biasrow = cbp.tile([1, K], f32r, tag="biasrow")
                cbgb = cbp.tile([128, 4, D], bf16, tag="cbgb")
                # small codebook tensors first: the first matmul needs them
                nc.sync.dma_start(ct[:], ct_ext[b])
                nc.sync.dma_start(cdr[:], cdr_ext[b])
                nc.sync.dma_start(biasrow[:], biasrow_ext[b])
                nc.sync.dma_start(cbgb[:], cbgb_ext[b])
                nc.sync.dma_start(z[:], z_ext[b])
                nc.sync.dma_start(zdr[:], zdr_ext[b])

                out_sb = outp.tile([128, nt, D], bf16, tag="out")

                # two n-tiles per iteration: halves the per-op overhead on the
                # elementwise engines (psS spans 2 PSUM banks, one slice each)
                for p in range(nt // 2):
                    psS = psSp.tile([128, 2, K], f32, tag="psS")
                    for h in range(2):
                        n0 = (2 * p + h) * 128
                        # PSUM <- bias hi, then z1.c1 (f32r, exact), then the
                        # fp8 DoubleRow correction pass (z2.c1 + z1.c2 + bias lo)
                        nc.tensor.matmul(psS[:, h, :], ones[:], biasrow[:],
                                         start=True, stop=False)
                        for cd in range(2):
                            nc.tensor.matmul(psS[:, h, :], z[:, cd, n0:n0 + 128],
                                             ct[:, cd, :], start=False, stop=False)
                        for cd in range(2):
                            nc.tensor.matmul(
                                psS[:, h, :], zdr[:, cd, :, n0:n0 + 128],
                                cdr[:, cd, :, :], start=False, stop=(cd == 1),
                                perf_mode=mybir.MatmulPerfMode.DoubleRow)

                    # mx[n, h] = max_k S[n, h, k], straight from PSUM
                    mx = sco.tile([128, 2], f32, tag="mx")
                    nc.vector.reduce_max(mx[:], psS[:], axis=mybir.AxisListType.X)

                    # one-hot of the argmax, bf16
                    oh = sco.tile([128, 2, K], bf16, tag="oh")
                    for h in range(2):
                        nc.vector.tensor_scalar(
                            out=oh[:, h, :], in0=psS[:, h, :],
                            scalar1=mx[:, h:h + 1], scalar2=None,
                            op0=Alu.is_equal,
                        )

                    # transpose one-hot to [k, n] layout for the gather matmul
                    psT = psTp.tile([128, 2, K], bf16, tag="psT")
                    for h in range(2):
                        for j in range(4):
                            k0 = j * 128
                            nc.tensor.transpose(psT[:, h, k0:k0 + 128],
                                                oh[:, h, k0:k0 + 128], ident[:])
                    ohT = sco.tile([128, 2, K], bf16, tag="ohT")
                    nc.scalar.copy(ohT[:], psT[:])
                    psQ = psQp.tile([128, 2, D], f32, tag="psQ")
                    for h in range(2):
                        for j in range(4):
                            k0 = j * 128
                            nc.tensor.matmul(psQ[:, h, :],
                                             ohT[:, h, k0:k0 + 128],
                                             cbgb[:, j, :], start=(j == 0),
                                             stop=(j == 3))
                    nc.scalar.copy(out_sb[:, 2 * p:2 * p + 2, :], psQ[:])

                nc.sync.dma_start(out_ext[b], out_sb[:])

    nc.compile()
    return nc


def _get_nc():
    if "nc" not in _CACHE:
        _CACHE["nc"] = _build()
    return _CACHE["nc"]


def _r2n11(x):
    # Round f32 mantissas to 11 bits (nearest-even): the PE's float32r read
    # path keeps exactly 11 mantissa bits (hw-measured), so pre-rounded
    # operands make the f32r matmul exact.
    u = x.view(np.uint32).astype(np.uint64)
    rnd = np.uint64((1 << 11) - 1) + ((u >> np.uint64(12)) & np.uint64(1))
    return ((u + rnd) & ~np.uint64((1 << 12) - 1)).astype(np.uint32).view(np.float32)


def _prep_in_maps(z_e_x, c, embedding):
    import ml_dtypes

    bf = ml_dtypes.bfloat16
    f8 = ml_dtypes.float8_e4m3

    z = np.ascontiguousarray(np.asarray(z_e_x), dtype=np.float32)      # [B, D, H, W]
    cls = np.asarray(c).astype(np.int64)                               # [B]
    emb = np.ascontiguousarray(np.asarray(embedding), dtype=np.float32)

    # z in d-major [B, 2, 128, N] (d = cd*128 + p); main operand is
    # f16(z*2^6) (exact in the PE's e10m11 upcast), residual z2 = z - z1
    zf = np.ascontiguousarray(
        z.reshape(B, 2, 128, HH * WW))                                 # [B,2,128,N]
    z1s = (zf * np.float32(2.0 ** 6)).astype(np.float16)
    z1 = z1s.astype(np.float32) * np.float32(2.0 ** -6)
    z2 = zf - z1
    zr = np.ascontiguousarray(z1s.transpose(0, 2, 1, 3))
    # DR pairs: j=0 -> z2*2^9 (vs c1*2^9), j=1 -> z1*2^4 (vs c2*2^14)
    zdr = np.empty((B, 2, 2, 128, N), dtype=f8)
    zdr[:, :, 0] = (z2 * np.float32(2.0 ** 9)).astype(f8)
    zdr[:, :, 1] = (z1 * np.float32(2.0 ** 4)).astype(f8)

    cb = emb.reshape(NUM_CLASSES, K, D)[cls]                           # [B, 512, 256]
    cbT = np.ascontiguousarray(cb.transpose(0, 2, 1)).reshape(B, 2, 128, K)
    c1s = (cbT * np.float32(2.0 ** 12)).astype(np.float16)
    c1 = c1s.astype(np.float32) * np.float32(2.0 ** -12)
    c2 = cbT - c1
    ct = np.ascontiguousarray(c1s.transpose(0, 2, 1, 3))
    cdr = np.empty((B, 2, 2, 128, K), dtype=f8)
    cdr[:, :, 0] = (c1 * np.float32(2.0 ** 9)).astype(f8)
    cdr[:, :, 1] = (c2 * np.float32(2.0 ** 14)).astype(f8)

    bias = (-0.5 * 2.0 ** 18) * np.sum(cb.astype(np.float64) ** 2, axis=2)
    b11 = _r2n11(bias.astype(np.float32))                              # [B, 512]
    blo = (bias - b11.astype(np.float64)).astype(np.float32)
    # steal DR slot (cd=1, p=127, j=0): ones row on the z side, bias-lo on
    # the c side; drops the (tiny) z2.c1 contribution of d=255
    zdr[:, 1, 0, 127, :] = np.float32(1.0)
    cdr[:, 1, 0, 127, :] = blo.astype(f8)

    zdr = np.ascontiguousarray(zdr.transpose(0, 3, 1, 2, 4))           # [B,128,2,2,N]
    cdr = np.ascontiguousarray(cdr.transpose(0, 3, 1, 2, 4))           # [B,128,2,2,K]

    # gather operand: [B, 128, 4, 256] bf16 with k = j*128 + p
    cbgb = np.ascontiguousarray(
        cb.astype(bf).reshape(B, 4, 128, D).transpose(0, 2, 1, 3))

    biasrow = np.ascontiguousarray(b11[:, None, :])                    # [B, 1, 512]
    ones = np.ones((1, 128), dtype=np.float32)

    in_maps = []
    for i in range(NCORES):
        s = slice(i * BPC, (i + 1) * BPC)
        in_maps.append({
            "z": zr[s], "ct": ct[s], "zdr": zdr[s], "cdr": cdr[s],
            "cbgb": cbgb[s], "biasrow": biasrow[s], "ones": ones,
        })
    return in_maps


def kernel(z_e_x, c, embedding):
    from concourse.bass_utils import run_bass_kernel_spmd

    global LAST_EXEC_NS

    in_maps = _prep_in_maps(z_e_x, c, embedding)
    nc = _get_nc()
    res = run_bass_kernel_spmd(nc, in_maps, core_ids=list(range(NCORES)),
                               trace=TRACE)
    LAST_EXEC_NS = res.exec_time_ns

    outs = np.concatenate([res.results[i]["out"].astype(np.float32)
                           for i in range(NCORES)], axis=0)
    # [B, 128, NT, D] -> [B, N, D] with n = t*128 + p
    out = outs.transpose(0, 2, 1, 3).reshape(B, N, D)
    return np.ascontiguousarray(out.reshape(B, HH, WW, D))


# revision 13
# speedup vs baseline: 2.0781x; 2.0781x over previous
"""VQ codebook-lookup kernel for TRN2, data-parallel over batch on 8 NeuronCores.

Reference computation (per batch b with class c[b]):
  z = z_e_x[b] viewed as [N=4096, D=256] (D innermost)
  cb = embedding[c[b]*512:(c[b]+1)*512]            # [K=512, D]
  idx[n] = argmin_k ||z[n] - cb[k]||^2 = argmax_k (z[n].cb[k] - ||cb[k]||^2/2)
  out[n] = cb[idx[n]]

Device strategy per core (4 batches):
  - scores S[n,k] on TensorE, all at a common 2^18 product scale (argmax is
    scale-invariant):
      * main term z1.c1 in float32r (the PE keeps 11 mantissa bits of f32r
        operands, measured; operands are pre-rounded to 11 bits on the host
        so the pass is exact) — 2 matmuls over the d=256 contraction
      * cross corrections z2.c1 + z1.c2 in one fp8-e4m3 DoubleRow pair per
        PE cell — 2 matmuls; one DR slot is stolen to add the bias low part
      * bias high part (r2n11 of -2^18*||cb||^2/2) via a contraction-1
        f32r matmul that starts the PSUM accumulation group
  - row-max on DVE straight from PSUM, one-hot = (S == max) on DVE
  - one-hot transposed on TensorE, then a bf16 matmul gathers the codewords
Host side only reindexes/splits/quantizes operands and reassembles the
output.  Simulated end-to-end argmin flips vs the f32 reference: 2 of 131072
(the exact-f32 baseline had 4).
"""

import sys

sys.path.insert(0, "/opt/trn_rl_repo")

import numpy as np

B, D, HH, WW = 32, 256, 64, 64
N = HH * WW            # 4096 positions per batch
K = 512                # codes per class
NUM_CLASSES = 60
NCORES = 8
BPC = B // NCORES      # batches per core
NT = N // 128          # 32 n-tiles per batch

_CACHE = {}

# main-pass dtype: "f32r" (11-bit operands, 4B) or "bf16" (8-bit, 2B, faster)
MAIN_DT = "bf16"

# set by test harness to request an NTFF profile
TRACE = False
LAST_EXEC_NS = None


def _build(bpc=BPC, nt=NT, repeat=1, psum_cfg=(3, 1, 1), sco_bufs=3, zb_bufs=2,
           main_dt=None):
    from concourse import bacc, tile, mybir
    import ml_dtypes

    f32 = mybir.dt.float32
    f32r = mybir.dt.float32r
    f16 = mybir.dt.float16
    bf16 = mybir.dt.bfloat16
    Alu = mybir.AluOpType

    nc = bacc.Bacc("TRN2", target_bir_lowering=False)

    fp8 = mybir.dt.float8e4
    mdt = {"f32r": f32r, "bf16": bf16}[main_dt or MAIN_DT]
    z_ext = nc.declare_dram_parameter("z", [bpc, 128, 2, N], mdt, isOutput=False)
    ct_ext = nc.declare_dram_parameter("ct", [bpc, 128, 2, K], mdt, isOutput=False)
    zdr_ext = nc.declare_dram_parameter("zdr", [bpc, 128, 2, 2, N], fp8,
                                        isOutput=False)
    cdr_ext = nc.declare_dram_parameter("cdr", [bpc, 128, 2, 2, K], fp8,
                                        isOutput=False)
    cbgb_ext = nc.declare_dram_parameter("cbgb", [bpc, 128, 4, D], bf16,
                                         isOutput=False)
    biasrow_ext = nc.declare_dram_parameter("biasrow", [bpc, 2, K], bf16,
                                            isOutput=False)
    ones_ext = nc.declare_dram_parameter("ones", [2, 128], bf16, isOutput=False)
    out_ext = nc.declare_dram_parameter("out", [bpc, 128, nt, D], bf16, isOutput=True)

    ident_dram = nc.inline_tensor(np.eye(128, dtype=ml_dtypes.bfloat16), name="ident")

    with tile.TileContext(nc) as tc:
        with (
            tc.tile_pool(name="const", bufs=1) as constp,
            tc.tile_pool(name="zb", bufs=zb_bufs) as zb,
            tc.tile_pool(name="cbp", bufs=2) as cbp,
            tc.tile_pool(name="outp", bufs=2) as outp,
            tc.tile_pool(name="sco", bufs=sco_bufs) as sco,
            tc.tile_pool(name="psS", bufs=psum_cfg[0], space="PSUM") as psSp,
            tc.tile_pool(name="psT", bufs=psum_cfg[1], space="PSUM") as psTp,
            tc.tile_pool(name="psQ", bufs=psum_cfg[2], space="PSUM") as psQp,
        ):
            ident = constp.tile([128, 128], bf16, tag="ident")
            ones = constp.tile([2, 128], bf16, tag="ones")
            nc.sync.dma_start(ident[:], ident_dram[:])
            nc.sync.dma_start(ones[:], ones_ext[:])

            for b in [bb for _ in range(repeat) for bb in range(bpc)]:
                z = zb.tile([128, 2, N], mdt, tag="z")
                zdr = zb.tile([128, 2, 2, N], fp8, tag="zdr")
                ct = cbp.tile([128, 2, K], mdt, tag="ct")
                cdr = cbp.tile([128, 2, 2, K], fp8, tag="cdr")
                biasrow = cbp.tile([2, K], bf16, tag="biasrow")
                cbgb = cbp.tile([128, 4, D], bf16, tag="cbgb")
                # small codebook tensors first: the first matmul needs them
                nc.sync.dma_start(ct[:], ct_ext[b])
                nc.sync.dma_start(cdr[:], cdr_ext[b])
                nc.sync.dma_start(biasrow[:], biasrow_ext[b])
                nc.sync.dma_start(cbgb[:], cbgb_ext[b])
                nc.sync.dma_start(z[:], z_ext[b])
                nc.sync.dma_start(zdr[:], zdr_ext[b])

                out_sb = outp.tile([128, nt, D], bf16, tag="out")

                # two n-tiles per iteration: halves the per-op overhead on the
                # elementwise engines (psS spans 2 PSUM banks, one slice each)
                for p in range(nt // 2):
                    psS = psSp.tile([128, 2, K], f32, tag="psS")
                    for h in range(2):
                        n0 = (2 * p + h) * 128
                        # PSUM <- bias hi, then z1.c1 (f32r, exact), then the
                        # fp8 DoubleRow correction pass (z2.c1 + z1.c2 + bias lo)
                        nc.tensor.matmul(psS[:, h, :], ones[:], biasrow[:],
                                         start=True, stop=False)
                        for cd in range(2):
                            nc.tensor.matmul(psS[:, h, :], z[:, cd, n0:n0 + 128],
                                             ct[:, cd, :], start=False, stop=False)
                        for cd in range(2):
                            nc.tensor.matmul(
                                psS[:, h, :], zdr[:, cd, :, n0:n0 + 128],
                                cdr[:, cd, :, :], start=False, stop=(cd == 1),
                                perf_mode=mybir.MatmulPerfMode.DoubleRow)

                    # mx[n, h] = max_k S[n, h, k], straight from PSUM
                    mx = sco.tile([128, 2], f32, tag="mx")
                    nc.vector.reduce_max(mx[:], psS[:], axis=mybir.AxisListType.X)

                    # one-hot of the argmax, bf16
                    oh = sco.tile([128, 2, K], bf16, tag="oh")
                    for h in range(2):
                        nc.vector.tensor_scalar(
                            out=oh[:, h, :], in0=psS[:, h, :],
                            scalar1=mx[:, h:h + 1], scalar2=None,
                            op0=Alu.is_equal,
                        )

                    # transpose one-hot to [k, n] layout for the gather matmul
                    psT = psTp.tile([128, 2, K], bf16, tag="psT")
                    for h in range(2):
                        for j in range(4):
                            k0 = j * 128
                            nc.tensor.transpose(psT[:, h, k0:k0 + 128],
                                                oh[:, h, k0:k0 + 128], ident[:])
                    ohT = sco.tile([128, 2, K], bf16, tag="ohT")
                    nc.scalar.copy(ohT[:], psT[:])
                    psQ = psQp.tile([128, 2, D], f32, tag="psQ")
                    for h in range(2):
                        for j in range(4):
                            k0 = j * 128
                            nc.tensor.matmul(psQ[:, h, :],
                                             ohT[:, h, k0:k0 + 128],
                                             cbgb[:, j, :], start=(j == 0),
                                             stop=(j == 3))
                    nc.scalar.copy(out_sb[:, 2 * p:2 * p + 2, :], psQ[:])

                nc.sync.dma_start(out_ext[b], out_sb[:])

    nc.compile()
    return nc


def _get_nc():
    if "nc" not in _CACHE:
        _CACHE["nc"] = _build()
    return _CACHE["nc"]


def _r2n11(x):
    # Round f32 mantissas to 11 bits (nearest-even): the PE's float32r read
    # path keeps exactly 11 mantissa bits (hw-measured), so pre-rounded
    # operands make the f32r matmul exact.
    u = x.view(np.uint32).astype(np.uint64)
    rnd = np.uint64((1 << 11) - 1) + ((u >> np.uint64(12)) & np.uint64(1))
    return ((u + rnd) & ~np.uint64((1 << 12) - 1)).astype(np.uint32).view(np.float32)


def _prep_in_maps(z_e_x, c, embedding):
    import ml_dtypes

    bf = ml_dtypes.bfloat16
    f8 = ml_dtypes.float8_e4m3

    z = np.ascontiguousarray(np.asarray(z_e_x), dtype=np.float32)      # [B, D, H, W]
    cls = np.asarray(c).astype(np.int64)                               # [B]
    emb = np.ascontiguousarray(np.asarray(embedding), dtype=np.float32)

    # z in d-major [B, 2, 128, N] (d = cd*128 + p), split z = z1 + z2.  All
    # score terms share the 2^18 product scale (argmax is scale-invariant).
    zf = np.ascontiguousarray(
        z.reshape(B, 2, 128, HH * WW))                                 # [B,2,128,N]
    cb = emb.reshape(NUM_CLASSES, K, D)[cls]                           # [B, 512, 256]
    cbT = np.ascontiguousarray(cb.transpose(0, 2, 1)).reshape(B, 2, 128, K)
    if MAIN_DT == "bf16":
        z1s = (zf * np.float32(2.0 ** 6)).astype(bf)
        z1 = z1s.astype(np.float32) * np.float32(2.0 ** -6)
        c1s = (cbT * np.float32(2.0 ** 12)).astype(bf)
        c1 = c1s.astype(np.float32) * np.float32(2.0 ** -12)
        zr = np.ascontiguousarray(z1s.transpose(0, 2, 1, 3))
        ct = np.ascontiguousarray(c1s.transpose(0, 2, 1, 3))
    else:
        z1 = _r2n11(zf)
        c1 = _r2n11(cbT)
        zr = np.ascontiguousarray((z1 * np.float32(2.0 ** 9)).transpose(0, 2, 1, 3))
        ct = np.ascontiguousarray((c1 * np.float32(2.0 ** 9)).transpose(0, 2, 1, 3))
    z2 = zf - z1
    c2 = cbT - c1
    # DR pairs: j=0 -> z2*2^10 (vs c1*2^8), j=1 -> z1*2^4 (vs c2*2^14)
    zdr = np.empty((B, 2, 2, 128, N), dtype=f8)
    zdr[:, :, 0] = (z2 * np.float32(2.0 ** 10)).astype(f8)
    zdr[:, :, 1] = (z1 * np.float32(2.0 ** 4)).astype(f8)
    cdr = np.empty((B, 2, 2, 128, K), dtype=f8)
    cdr[:, :, 0] = (c1 * np.float32(2.0 ** 8)).astype(f8)
    cdr[:, :, 1] = (c2 * np.float32(2.0 ** 14)).astype(f8)
    zdr = np.ascontiguousarray(zdr.transpose(0, 3, 1, 2, 4))           # [B,128,2,2,N]
    cdr = np.ascontiguousarray(cdr.transpose(0, 3, 1, 2, 4))           # [B,128,2,2,K]

    bias = (-0.5 * 2.0 ** 18) * np.sum(cb.astype(np.float64) ** 2, axis=2)
    bhi = bias.astype(np.float32).astype(bf)                           # [B, 512]
    blo = (bias - bhi.astype(np.float64)).astype(np.float32).astype(bf)

    # gather operand: [B, 128, 4, 256] bf16 with k = j*128 + p
    cbgb = np.ascontiguousarray(
        cb.astype(bf).reshape(B, 4, 128, D).transpose(0, 2, 1, 3))

    biasrow = np.ascontiguousarray(np.stack([bhi, blo], axis=1))       # [B, 2, 512]
    ones = np.ones((2, 128), dtype=ml_dtypes.bfloat16)

    in_maps = []
    for i in range(NCORES):
        s = slice(i * BPC, (i + 1) * BPC)
        in_maps.append({
            "z": zr[s], "ct": ct[s], "zdr": zdr[s], "cdr": cdr[s],
            "cbgb": cbgb[s], "biasrow": biasrow[s], "ones": ones,
        })
    return in_maps


def kernel(z_e_x, c, embedding):
    from concourse.bass_utils import run_bass_kernel_spmd

    global LAST_EXEC_NS

    in_maps = _prep_in_maps(z_e_x, c, embedding)
    nc = _get_nc()
    res = run_bass_kernel_spmd(nc, in_maps, core_ids=list(range(NCORES)),
                               trace=TRACE)
    LAST_EXEC_NS = res.exec_time_ns

    outs = np.concatenate([res.results[i]["out"].astype(np.float32)
                           for i in range(NCORES)], axis=0)
    # [B, 128, NT, D] -> [B, N, D] with n = t*128 + p
    out = outs.transpose(0, 2, 1, 3).reshape(B, N, D)
    return np.ascontiguousarray(out.reshape(B, HH, WW, D))
